# revision 7
# baseline (speedup 1.0000x reference)
"""LIF-with-residue Trainium2 kernel (v3).

Reference semantics (T=4, THRESH=1, TAU=1, ALPHA=0.5):
    x: [32, 1024, 512] fp32 -> flat timeline [128 steps, 256, 512]
    per step t:
        mem   = mem + x_t
        sp    = (mem >= 1.0)
        res   = 0.5 * res + sp          # output at step t
        mem   = mem * (1 - sp)

Design (per core, 16384 neurons = 128 partitions x 128 f, 128 steps):
  Two fused custom-DVE ops per step (registered at import):
    LIF_STEP_ANT:  memb_t = memb_{t-1} * (memb_{t-1} < 1) + x_t   (fp32,
                   bit-exact vs the reference)
    RES_STEP_ANT:  res_t  = res_{t-1} * 0.5 + (memb_t >= 1)       (bf16
                   state/output; ~0.5% rel err, no feedback into memb)
  ~480ns/step on the Vector engine; everything else (x in, res out) is
  chunked DMA fully overlapped with the chain. Residue output is
  t-major bf16, upcast on the host.

Sharding: neuron n_core = p*128 + f; core c owns neurons
[c*16384, (c+1)*16384) -- data-parallel, no cross-core comms.
"""

import numpy as np

N_STEPS = 128
N_NEURONS = 131072
N_CORES = 8
N_PER_CORE = N_NEURONS // N_CORES   # 16384
P = 128                             # SBUF partitions
F = N_PER_CORE // P                 # 128 neurons per partition
CHUNK = 32                          # steps per DMA chunk

_CACHE = {}


def _register_ops():
    """Register the two fused custom DVE ops (idempotent)."""
    import concourse.dve_ops as dve_ops
    from concourse.dve_spec import Spec, Src0, Src1, C0, C1, lower
    from concourse.dve_uop import DveOpSpec

    def reg(name, spec):
        for o in dve_ops.OPS:
            if o.name == name:
                return o
        row = max(dve_ops._SUB_OPCODE_FOR_NAME.values()) + 1
        assert row < 0x20
        shas = {}
        for ver in ("v3", "v4"):
            d = DveOpSpec(name=name, opcode=row, uops=lower(spec, ver=ver),
                          rd1_en=True)
            shas[ver] = d.sha(ver)
        op = dve_ops.DveOp(name, spec, subdim=False, uops_sha=shas)
        dve_ops.OPS.append(op)
        dve_ops.CUSTOM_DVE_SPECS[name] = spec
        dve_ops._SUB_OPCODE_FOR_NAME[name] = row
        return op

    lif = reg(
        "LIF_STEP_ANT",
        Spec(
            body=Src0 * (Src0 < C0) + Src1,
            reference=lambda in0, in1, s0, s1, imm2: (
                in0 * (in0 < s0) + in1
            ).astype(np.float32),
        ),
    )
    res = reg(
        "RES_STEP_ANT",
        Spec(
            body=Src0 * C0 + (Src1 >= C1),
            reference=lambda in0, in1, s0, s1, imm2: (
                in0 * s0 + (in1 >= s1)
            ).astype(np.float32),
        ),
    )
    return lif, res


def _build_program():
    import concourse.bacc as bacc
    import concourse.mybir as mybir
    from concourse.tile import TileContext

    f32 = mybir.dt.float32
    bf16 = mybir.dt.bfloat16
    lif, res = _register_ops()

    nc = bacc.Bacc()
    x_d = nc.dram_tensor("x", [P, N_STEPS * F], f32, kind="ExternalInput")
    o_d = nc.dram_tensor("o", [P, N_STEPS * F], bf16, kind="ExternalOutput")

    # Variable chunking: small first chunk cuts the initial DMA wait,
    # small last chunks cut the output tail.
    in_chunks = [(0, 2), (2, 6), (8, 24), (32, 32), (64, 32), (96, 32)]
    out_chunks = [(0, 16), (16, 16), (32, 32), (64, 32), (96, 16),
                  (112, 12), (124, 4)]

    with TileContext(nc) as tc:
        with (
            tc.tile_pool(name="xin", bufs=2) as xpool,
            tc.tile_pool(name="memb", bufs=3) as mpool,
            tc.tile_pool(name="rout", bufs=2) as rpool,
            tc.tile_pool(name="single", bufs=1) as spool,
        ):
            zero_m = spool.tile([P, F], f32)
            zero_r = spool.tile([P, F], bf16)
            nc.vector.memset(zero_m[:], 0.0)
            nc.vector.memset(zero_r[:], 0.0)

            in_iter = iter(in_chunks)
            out_iter = iter(out_chunks)
            xt = x0 = xlen = rt = r0 = rlen = None
            m_prev = zero_m[:]
            r_prev = zero_r[:]
            mb_hold = {}
            rt_of = {}

            def emit_res(t):
                """Emit RES for step t (software-pipelined one step behind
                LIF so no op consumes the previous op's output on Src1)."""
                nonlocal r_prev
                rtile, base, ln = rt_of.pop(t)
                rcol = rtile[:, t - base, :]
                nc.vector._custom_dve(
                    res, out=rcol, in0=r_prev, in1=mb_hold.pop(t)[:],
                    s0=0.5, s1=1.0,
                )
                r_prev = rcol
                if t == base + ln - 1:
                    nc.sync.dma_start(
                        out=o_d[:, base * F:(base + ln) * F], in_=rtile[:]
                    )

            for t in range(N_STEPS):
                if xt is None or t - x0 >= xlen:
                    x0, xlen = next(in_iter)
                    xt = xpool.tile([P, xlen, F], f32)
                    nc.sync.dma_start(
                        out=xt[:], in_=x_d[:, x0 * F:(x0 + xlen) * F]
                    )
                if rt is None or t - r0 >= rlen:
                    r0, rlen = next(out_iter)
                    rt = rpool.tile([P, rlen, F], bf16)
                rt_of[t] = (rt, r0, rlen)

                mb = mpool.tile([P, F], f32)
                nc.vector._custom_dve(
                    lif, out=mb[:], in0=m_prev, in1=xt[:, t - x0, :], s0=1.0,
                )
                m_prev = mb[:]
                mb_hold[t] = mb
                if t >= 1:
                    emit_res(t - 1)
            emit_res(N_STEPS - 1)
    nc.finalize()
    return nc


def _get_program():
    if "nc" not in _CACHE:
        _CACHE["nc"] = _build_program()
    return _CACHE["nc"]


def _shard_inputs(x: np.ndarray) -> list[np.ndarray]:
    """[32,1024,512] -> per-core [P, N_STEPS*F] partition-major arrays."""
    xf = np.ascontiguousarray(x, dtype=np.float32).reshape(N_STEPS, N_NEURONS)
    shards = []
    for c in range(N_CORES):
        s = xf[:, c * N_PER_CORE:(c + 1) * N_PER_CORE]   # [T, 16384]
        s = s.reshape(N_STEPS, P, F).transpose(1, 0, 2).reshape(
            P, N_STEPS * F
        )
        shards.append(np.ascontiguousarray(s))
    return shards


def _unshard_outputs(outs: list[np.ndarray]) -> np.ndarray:
    """Per-core o [P, T*F] bf16 (t-major) -> [32,1024,512] f32."""
    full = np.empty((N_STEPS, N_NEURONS), dtype=np.float32)
    for c, o in enumerate(outs):
        s = np.asarray(o).astype(np.float32).reshape(P, N_STEPS, F)
        full[:, c * N_PER_CORE:(c + 1) * N_PER_CORE] = (
            s.transpose(1, 0, 2).reshape(N_STEPS, N_PER_CORE)
        )
    return full.reshape(32, 1024, 512)


def kernel(x: np.ndarray) -> np.ndarray:
    from concourse.bass_utils import run_bass_kernel_spmd

    steps, tb, d = x.shape
    assert (steps, tb, d) == (32, 1024, 512), x.shape

    in_maps = [{"x": s} for s in _shard_inputs(x)]
    nc = _get_program()
    res = run_bass_kernel_spmd(nc, in_maps, list(range(N_CORES)))
    return _unshard_outputs(
        [res.results[c]["o"] for c in range(N_CORES)]
    )


# revision 8
# speedup vs baseline: 1.0173x; 1.0173x over previous
"""LIF-with-residue Trainium2 kernel (v3).

Reference semantics (T=4, THRESH=1, TAU=1, ALPHA=0.5):
    x: [32, 1024, 512] fp32 -> flat timeline [128 steps, 256, 512]
    per step t:
        mem   = mem + x_t
        sp    = (mem >= 1.0)
        res   = 0.5 * res + sp          # output at step t
        mem   = mem * (1 - sp)

Design (per core, 16384 neurons = 128 partitions x 128 f, 128 steps):
  Two fused custom-DVE ops per step (registered at import):
    LIF_STEP_ANT:  memb_t = memb_{t-1} * (memb_{t-1} < 1) + x_t   (fp32,
                   bit-exact vs the reference)
    RES_STEP_ANT:  res_t  = res_{t-1} * 0.5 + (memb_t >= 1)       (bf16
                   state/output; ~0.5% rel err, no feedback into memb)
  ~480ns/step on the Vector engine; everything else (x in, res out) is
  chunked DMA fully overlapped with the chain. Residue output is
  t-major bf16, upcast on the host.

Sharding: neuron n_core = p*128 + f; core c owns neurons
[c*16384, (c+1)*16384) -- data-parallel, no cross-core comms.
"""

import numpy as np

N_STEPS = 128
N_NEURONS = 131072
N_CORES = 8
N_PER_CORE = N_NEURONS // N_CORES   # 16384
P = 128                             # SBUF partitions
F = N_PER_CORE // P                 # 128 neurons per partition
CHUNK = 32                          # steps per DMA chunk

_CACHE = {}


def _register_ops():
    """Register the two fused custom DVE ops (idempotent)."""
    import concourse.dve_ops as dve_ops
    from concourse.dve_spec import Spec, Src0, Src1, C0, C1, lower
    from concourse.dve_uop import DveOpSpec

    def reg(name, spec):
        for o in dve_ops.OPS:
            if o.name == name:
                return o
        row = max(dve_ops._SUB_OPCODE_FOR_NAME.values()) + 1
        assert row < 0x20
        shas = {}
        for ver in ("v3", "v4"):
            d = DveOpSpec(name=name, opcode=row, uops=lower(spec, ver=ver),
                          rd1_en=True)
            shas[ver] = d.sha(ver)
        op = dve_ops.DveOp(name, spec, subdim=False, uops_sha=shas)
        dve_ops.OPS.append(op)
        dve_ops.CUSTOM_DVE_SPECS[name] = spec
        dve_ops._SUB_OPCODE_FOR_NAME[name] = row
        return op

    lif = reg(
        "LIF_STEP_ANT",
        Spec(
            body=Src0 * (Src0 < C0) + Src1,
            reference=lambda in0, in1, s0, s1, imm2: (
                in0 * (in0 < s0) + in1
            ).astype(np.float32),
        ),
    )
    res = reg(
        "RES_STEP_ANT",
        Spec(
            body=Src0 * C0 + (Src1 >= C1),
            reference=lambda in0, in1, s0, s1, imm2: (
                in0 * s0 + (in1 >= s1)
            ).astype(np.float32),
        ),
    )
    return lif, res


def _build_program():
    import concourse.bacc as bacc
    import concourse.mybir as mybir
    from concourse.tile import TileContext

    f32 = mybir.dt.float32
    bf16 = mybir.dt.bfloat16
    lif, res = _register_ops()

    nc = bacc.Bacc()
    x_d = nc.dram_tensor("x", [P, N_STEPS * F], f32, kind="ExternalInput")
    o_d = nc.dram_tensor("o", [P, N_STEPS * F], bf16, kind="ExternalOutput")

    # Variable chunking: small first chunk cuts the initial DMA wait,
    # small last chunks cut the output tail.
    in_chunks = [(0, 8), (8, 24), (32, 32), (64, 32), (96, 32)]
    out_chunks = [(0, 16), (16, 16), (32, 32), (64, 32), (96, 16), (112, 16)]

    with TileContext(nc) as tc:
        with (
            tc.tile_pool(name="xin", bufs=2) as xpool,
            tc.tile_pool(name="memb", bufs=3) as mpool,
            tc.tile_pool(name="rout", bufs=2) as rpool,
            tc.tile_pool(name="single", bufs=1) as spool,
        ):
            zero_m = spool.tile([P, F], f32)
            zero_r = spool.tile([P, F], bf16)
            nc.vector.memset(zero_m[:], 0.0)
            nc.vector.memset(zero_r[:], 0.0)

            in_iter = iter(in_chunks)
            out_iter = iter(out_chunks)
            xt = x0 = xlen = rt = r0 = rlen = None
            m_prev = zero_m[:]
            r_prev = zero_r[:]
            mb_hold = {}
            rt_of = {}

            def emit_res(t):
                """Emit RES for step t (software-pipelined one step behind
                LIF so no op consumes the previous op's output on Src1)."""
                nonlocal r_prev
                rtile, base, ln = rt_of.pop(t)
                rcol = rtile[:, t - base, :]
                nc.vector._custom_dve(
                    res, out=rcol, in0=r_prev, in1=mb_hold.pop(t)[:],
                    s0=0.5, s1=1.0,
                )
                r_prev = rcol
                if t == base + ln - 1:
                    nc.sync.dma_start(
                        out=o_d[:, base * F:(base + ln) * F], in_=rtile[:]
                    )

            for t in range(N_STEPS):
                if xt is None or t - x0 >= xlen:
                    x0, xlen = next(in_iter)
                    xt = xpool.tile([P, xlen, F], f32)
                    nc.sync.dma_start(
                        out=xt[:], in_=x_d[:, x0 * F:(x0 + xlen) * F]
                    )
                if rt is None or t - r0 >= rlen:
                    r0, rlen = next(out_iter)
                    rt = rpool.tile([P, rlen, F], bf16)
                rt_of[t] = (rt, r0, rlen)

                mb = mpool.tile([P, F], f32)
                nc.vector._custom_dve(
                    lif, out=mb[:], in0=m_prev, in1=xt[:, t - x0, :], s0=1.0,
                )
                m_prev = mb[:]
                mb_hold[t] = mb
                if t >= 1:
                    emit_res(t - 1)
            emit_res(N_STEPS - 1)
    nc.finalize()
    return nc


def _get_program():
    if "nc" not in _CACHE:
        _CACHE["nc"] = _build_program()
    return _CACHE["nc"]


def _shard_inputs(x: np.ndarray) -> list[np.ndarray]:
    """[32,1024,512] -> per-core [P, N_STEPS*F] partition-major arrays."""
    xf = np.ascontiguousarray(x, dtype=np.float32).reshape(N_STEPS, N_NEURONS)
    shards = []
    for c in range(N_CORES):
        s = xf[:, c * N_PER_CORE:(c + 1) * N_PER_CORE]   # [T, 16384]
        s = s.reshape(N_STEPS, P, F).transpose(1, 0, 2).reshape(
            P, N_STEPS * F
        )
        shards.append(np.ascontiguousarray(s))
    return shards


def _unshard_outputs(outs: list[np.ndarray]) -> np.ndarray:
    """Per-core o [P, T*F] bf16 (t-major) -> [32,1024,512] f32."""
    full = np.empty((N_STEPS, N_NEURONS), dtype=np.float32)
    for c, o in enumerate(outs):
        s = np.asarray(o).astype(np.float32).reshape(P, N_STEPS, F)
        full[:, c * N_PER_CORE:(c + 1) * N_PER_CORE] = (
            s.transpose(1, 0, 2).reshape(N_STEPS, N_PER_CORE)
        )
    return full.reshape(32, 1024, 512)


def kernel(x: np.ndarray) -> np.ndarray:
    from concourse.bass_utils import run_bass_kernel_spmd

    steps, tb, d = x.shape
    assert (steps, tb, d) == (32, 1024, 512), x.shape

    in_maps = [{"x": s} for s in _shard_inputs(x)]
    nc = _get_program()
    res = run_bass_kernel_spmd(nc, in_maps, list(range(N_CORES)))
    return _unshard_outputs(
        [res.results[c]["o"] for c in range(N_CORES)]
    )
